# revision 32
# baseline (speedup 1.0000x reference)
"""Multi-head attention (quirky Dense(d_k) variant) on 8 trn2 NeuronCores.

Sharding: data-parallel over batch (B=2), tensor-parallel over heads
(8 heads -> 4 groups of 2 heads). Core c: batch c//4, head-group c%4.
Each core is fully independent (no collectives); host sums the 4 partial
outputs per batch (Wo row-sharded -> partial sums) and adds bo.

All matmul operands are bf16 (f32 PSUM accumulation); inputs/weights are
converted to bf16 on the host, halving HBM traffic and running the PE at
full rate. The two heads' 16 dims sit at partitions 0-15 and 32-47
(32-aligned bases) so score matmuls row-tile-pack (tile_position (0,0) /
(32,0)); k/v and q-half projections col-tile-pack (out partitions 0-47 /
64-111 of one PSUM bank) so two projection streams run concurrently.

Per-core dataflow (L=2048, d_model=1024), two passes over q halves:
  qT/kT/vT = W48^T @ X^T        (PE bf16 col-packed; bias folded into the
                                 DVE PSUM->SBUF copy as tensor_scalar_add)
  v        = transpose(vT)      (PE transpose, per 128-key tile)
  s        = k_h^T . q_h        (PE, [128 k x 1024 q], 2 heads row-packed)
  e        = exp(s * scale)     (ACT, PSUM f32 -> SBUF bf16; no max sub)
  oT/rsum  = [v|1]^T @ e        (PE, PSUM [49, 1024], rsum via ones col)
  rrec     = 1/rsum             (DMA transpose to [128,16], DVE reciprocal,
                                 DMA back to rows, PE K=1 broadcast to rb)
  oTn      = oT * rb            (DVE, normalized bf16 oT; rows 16-31 zero)
  y        = oTn^T @ Wo         (PE K=48; Wo rows 16-31 zero on host)
"""

import math
import sys

sys.path.insert(0, "/opt/trn_rl_repo")

import numpy as np

import concourse.bass as bass
import concourse.mybir as mybir
import concourse.tile as tile
from concourse import bacc
from concourse.bass_utils import run_bass_kernel_spmd

H = 8
D_MODEL = 1024
D_K = 128          # projection width (d_model / h)
HD = 16            # per-head dim after reshape
B, L = 2, 2048
DSL = 32           # per-core slice of D_K (2 heads x 16)
DP = 48            # padded: head0 dims at 0-15, head1 at 32-47
CC = 8             # contraction chunks of 128 over d_model
LQ = 1024          # queries per pass
NP = 2             # q passes
NLB = 4            # k/v blocks
LB = L // NLB      # 512
SCALE = 1.0 / math.sqrt(float(D_K))   # reference scales by sqrt(d_k)=sqrt(128)
F32 = mybir.dt.float32
BF16 = mybir.dt.bfloat16

_CACHE = {}


def _build_nc():
    nc = bacc.Bacc(None, target_bir_lowering=False)

    xq = nc.declare_dram_parameter("xq_t", [D_MODEL, L], BF16, isOutput=False)
    xk = nc.declare_dram_parameter("xk_t", [D_MODEL, L], BF16, isOutput=False)
    xv = nc.declare_dram_parameter("xv_t", [D_MODEL, L], BF16, isOutput=False)
    # host-packed [128, CC*DP]: partition p, chunk c, out-dim d
    wq = nc.declare_dram_parameter("wq", [128, CC * DP], BF16, isOutput=False)
    wk = nc.declare_dram_parameter("wk", [128, CC * DP], BF16, isOutput=False)
    wv = nc.declare_dram_parameter("wv", [128, CC * DP], BF16, isOutput=False)
    bqkv = nc.declare_dram_parameter("bqkv", [DP, 3], F32, isOutput=False)
    # rows 0-15: Wo head0; 32-47: Wo head1; 16-31 zero (bo added on host)
    wo = nc.declare_dram_parameter("wo", [DP, D_MODEL], BF16, isOutput=False)
    identp = nc.declare_dram_parameter("identp", [DP, DP], BF16, isOutput=False)
    y = nc.declare_dram_parameter("y", [L, D_MODEL], BF16, isOutput=True)
    import os
    dbg = os.environ.get("KERNEL_DEBUG", "0") == "1"
    if dbg:
        qt_d = nc.declare_dram_parameter("qt_d", [DP, L], BF16, isOutput=True)
        kt_d = nc.declare_dram_parameter("kt_d", [DP, L], BF16, isOutput=True)
        vsb_d = nc.declare_dram_parameter("vsb_d", [128, 4 * 144], BF16, isOutput=True)
        ot_d = nc.declare_dram_parameter("ot_d", [DP, L], BF16, isOutput=True)
        stg_d = nc.declare_dram_parameter("stg_d", [49, L], F32, isOutput=True)
        rb_d = nc.declare_dram_parameter("rb_d", [DP, L], F32, isOutput=True)
        e_d = nc.declare_dram_parameter("e_d", [128, 4 * LQ], BF16, isOutput=True)
        sc_d = nc.declare_dram_parameter("sc_d", [128, 4 * LQ], F32, isOutput=True)

    Exp = mybir.ActivationFunctionType.Exp
    Mult = mybir.AluOpType.mult

    with tile.TileContext(nc) as tc:
        with (
            tc.tile_pool(name="const", bufs=1) as constp,
            tc.tile_pool(name="xin", bufs=1) as xpool,
            tc.tile_pool(name="qk", bufs=1) as qkpool,
            tc.tile_pool(name="psS", bufs=1, space="PSUM") as psS,
            tc.tile_pool(name="psO", bufs=2, space="PSUM") as psO,
            tc.tile_pool(name="psA", bufs=2, space="PSUM") as psA,
            tc.tile_pool(name="ep", bufs=2) as epool,
            tc.tile_pool(name="yp", bufs=3) as ypool,
            tc.tile_pool(name="misc", bufs=2) as misc,
            tc.tile_pool(name="dr", bufs=2, space="DRAM") as drpool,
        ):
            # ---- constants (gpsimd DMA queue; x loads go on sync) ----
            wq_sb = constp.tile([128, CC, DP], BF16)
            nc.gpsimd.dma_start(out=wq_sb, in_=wq[:].rearrange("p (c d) -> p c d", c=CC))
            wk_sb = constp.tile([128, CC, DP], BF16)
            nc.gpsimd.dma_start(out=wk_sb, in_=wk[:].rearrange("p (c d) -> p c d", c=CC))
            wv_sb = constp.tile([128, CC, DP], BF16)
            nc.gpsimd.dma_start(out=wv_sb, in_=wv[:].rearrange("p (c d) -> p c d", c=CC))
            woh = constp.tile([DP, D_MODEL], BF16)
            nc.gpsimd.dma_start(out=woh, in_=wo[:])
            bias_sb = constp.tile([DP, 3], F32)
            nc.gpsimd.dma_start(out=bias_sb, in_=bqkv[:])
            ident = constp.tile([DP, DP], BF16)
            nc.gpsimd.dma_start(out=ident, in_=identp[:])
            # force the ACT exp table-set load off the critical path
            warm = constp.tile([1, 16], F32)
            nc.vector.memset(warm, 0.0)
            warm2 = constp.tile([1, 16], F32)
            nc.scalar.activation(warm2, warm, Exp, scale=1.0)

            # ---- resident x: one DMA per (tensor, d_model chunk) -- full-L
            # [128, 2048] slices are 4KB/partition contiguous, so each DMA is
            # 128 descriptors instead of 1024; projections stream per chunk ----
            xkc = [xpool.tile([128, L], BF16, name=f"xkc{_i}") for _i in range(CC)]
            xvc = [xpool.tile([128, L], BF16, name=f"xvc{_i}") for _i in range(CC)]
            xqc = [xpool.tile([128, L], BF16, name=f"xqc{_i}") for _i in range(CC)]
            for cc in range(CC):
                nc.sync.dma_start(out=xkc[cc], in_=xk[cc * 128:(cc + 1) * 128, :])
                nc.scalar.dma_start(out=xqc[cc], in_=xq[cc * 128:(cc + 1) * 128, :])
            for cc in range(CC):
                nc.gpsimd.dma_start(out=xvc[cc], in_=xv[cc * 128:(cc + 1) * 128, :])

            # qT4: col-packed projection layout kept as-is -- q-sub0 dims at
            # rows 0-47, q-sub1 at rows 64-111 (4 head/sub groups at 32k)
            qT = [qkpool.tile([112, LB], BF16, name=f"qT{_i}") for _i in range(NP)]
            # ktb4: k^T at rows 0-47 plus a copy at rows 64-111 for the
            # 4-way row-packed score matmuls
            ktb = [qkpool.tile([112, LB], BF16, name=f"ktb{_i}") for _i in range(NLB)]
            vtb = [qkpool.tile([DP, LB], BF16, name=f"vtb{_i}") for _i in range(NLB)]
            # v natural + ones col, per Lk-tile t within block (36 cols):
            # [16 v_h0 | 1 | pad | 16 v_h1 | 1 | pad]
            vsb = [constp.tile([128, 4 * 36], BF16, name=f"vsb{_i}") for _i in range(NLB)]
            for lb in range(NLB):
                v3 = vsb[lb].rearrange("p (t s) -> p t s", s=36)
                nc.vector.memset(v3[:, :, 16:17], 1.0)
                nc.vector.memset(v3[:, :, 34:35], 1.0)

            # oTn rows 0-15 / 32-47: normalized o^T per head (bf16);
            # rows 16-31 zeroed so the K=48 output matmul is junk-safe
            oT = [qkpool.tile([DP, LQ], BF16, name=f"oT{_i}") for _i in range(NP)]
            for p in range(NP):
                nc.vector.memset(oT[p], 0.0)
            stg = [misc.tile([113, 512], F32, tag=f"stg{_i}", name=f"stg{_i}")
                   for _i in range(NP)]
            rb_sb = [misc.tile([128, 512], F32, tag=f"rb{_i}", name=f"rb{_i}")
                     for _i in range(NP)]
            scratch = [drpool.tile([4, 512], F32, name=f"scr{_i}") for _i in range(NP)]

            import os

            def proj_pair(w0, x0, c0, out0, b0, w1, x1, c1, out1, b1):
                # two 48-wide projection streams col-packed in one PSUM bank:
                # stream0 -> partitions 0-47, stream1 -> partitions 64-111
                pb = psA.tile([112, LB], F32, tag="ps", name="pb")
                for cc in range(CC):
                    nc.tensor.matmul(
                        pb[0:DP, :], lhsT=w0[:, cc, :], rhs=x0[cc][:, c0:c0 + LB],
                        start=(cc == 0), stop=(cc == CC - 1),
                    )
                    nc.tensor.matmul(
                        pb[64:64 + DP, :], lhsT=w1[:, cc, :], rhs=x1[cc][:, c1:c1 + LB],
                        start=(cc == 0), stop=(cc == CC - 1),
                    )
                nc.vector.tensor_scalar_add(out0, pb[0:DP, :], bias_sb[:, b0:b0 + 1])
                nc.vector.tensor_scalar_add(out1, pb[64:64 + DP, :], bias_sb[:, b1:b1 + 1])

            def produce_kv(lb):
                proj_pair(
                    wk_sb, xkc, lb * LB, ktb[lb][0:DP, :], 1,
                    wv_sb, xvc, lb * LB, vtb[lb], 2,
                )
                nc.gpsimd.tensor_copy(ktb[lb][64:64 + DP, :], ktb[lb][0:DP, :])
                # transpose this block's v into natural layout
                pvt = psA.tile([128, 4 * DP], BF16, tag="ps", name=f"pvt{lb}")
                for i in range(4):
                    nc.tensor.transpose(
                        pvt[:, i * DP:(i + 1) * DP],
                        vtb[lb][:, i * 128:(i + 1) * 128],
                        ident,
                    )
                v3 = vsb[lb].rearrange("p (t s) -> p t s", s=36)
                pvt3 = pvt.rearrange("p (t s) -> p t s", s=DP)
                for h in (0, 1):
                    nc.vector.tensor_copy(
                        v3[:, :, 18 * h:18 * h + 16], pvt3[:, :, 32 * h:32 * h + 16]
                    )

            def produce_q(p):
                proj_pair(
                    wq_sb, xqc, p * LQ, qT[p][0:DP, :], 0,
                    wq_sb, xqc, p * LQ + 512, qT[p][64:64 + DP, :], 0,
                )

            def attn_tile(p, psoo, t):
                lb, ti = t // 4, t % 4
                sc, es = [], []
                for h in (0, 1):
                    sc.append(psS.tile([128, LQ], F32, tag=f"sc{h}", name=f"sc{h}"))
                    es.append(epool.tile([128, LQ], BF16, tag=f"e{h}", name=f"et{h}"))
                # 4 score matmuls run concurrently in 4 PE row groups:
                # group g = 2*sub + h at rows 32g (q/k copies pre-placed)
                for sub in (0, 1):
                    for h in (0, 1):
                        g = 64 * sub + 32 * h
                        nc.tensor.matmul(
                            sc[h][:, sub * 512:(sub + 1) * 512],
                            lhsT=ktb[lb][g:g + 16, ti * 128:(ti + 1) * 128],
                            rhs=qT[p][g:g + 16, :],
                            start=True, stop=True,
                            tile_position=(g, 0),
                        )
                for h in (0, 1):
                    nc.scalar.activation(es[h], sc[h], Exp, scale=SCALE)
                if dbg and t == 15:
                    for h in (0, 1):
                        off = (2 * p + h) * LQ
                        nc.sync.dma_start(out=e_d[:, off:off + LQ], in_=es[h])
                        cpt = misc.tile([128, LQ], F32, tag="scd", name="scd")
                        nc.vector.tensor_copy(cpt, sc[h])
                        nc.sync.dma_start(out=sc_d[:, off:off + LQ], in_=cpt)
                # 4 o-accum matmuls in 4 PE col groups into one PSUM bank:
                # psoo rows 32g..32g+16 for group g = 2*sub + h
                for h in (0, 1):
                    for sub in (0, 1):
                        g = 64 * sub + 32 * h
                        nc.tensor.matmul(
                            psoo[g:g + 17, :],
                            lhsT=vsb[lb][:, ti * 36 + 18 * h:ti * 36 + 18 * h + 17],
                            rhs=es[h][:, sub * 512:(sub + 1) * 512],
                            start=(t == 0), stop=(t == 15),
                            tile_position=(0, g),
                        )

            def normalize(p, psoo):
                # stage PSUM rows to SBUF f32 (frees psoo for the next pass);
                # one copy on ACT, one on DVE so they run concurrently
                nc.scalar.copy(stg[p][0:49, :], psoo[0:49, :])
                nc.vector.tensor_copy(stg[p][64:113, :], psoo[64:113, :])
                # rsum rows (16+32g) -> DRAM -> [16, 128] -> reciprocal
                for g in range(4):
                    nc.gpsimd.dma_start(
                        out=scratch[p][g:g + 1, :], in_=stg[p][32 * g + 16:32 * g + 17, :]
                    )
                rsT = misc.tile([16, 128], F32, tag="rsT", name="rsT")
                nc.sync.dma_start(
                    out=rsT, in_=scratch[p][:].rearrange("g (i p) -> (g i) p", p=128)
                )
                rrT = misc.tile([16, 128], F32, tag="rrT", name="rrT")
                nc.vector.reciprocal(rrT, rsT)
                # back to DRAM, then stride-0 broadcast to 16 partitions/group
                scr3 = drpool.tile([4, 512], F32, name=f"scr3_{p}")
                nc.gpsimd.dma_start(
                    out=scr3[:].rearrange("g (i p) -> (g i) p", p=128), in_=rrT
                )
                for g in range(4):
                    eng = nc.sync if g % 2 else nc.gpsimd
                    eng.dma_start(
                        out=rb_sb[p][32 * g:32 * g + 16, :],
                        in_=scr3[g:g + 1, :].to_broadcast((16, 512)),
                    )
                # normalized bf16 oT for the output projection; group g holds
                # head g%2, q-sub g//2 -> oT[32h:+16, 512*sub:+512]
                for g in range(4):
                    h, sub = g % 2, g // 2
                    eng = nc.vector if g % 2 else nc.gpsimd
                    eng.tensor_tensor(
                        out=oT[p][32 * h:32 * h + 16, sub * 512:(sub + 1) * 512],
                        in0=stg[p][32 * g:32 * g + 16, :],
                        in1=rb_sb[p][32 * g:32 * g + 16, :],
                        op=Mult,
                    )

            def y_tile(p, i, tail=False):
                # y[p*1024 + i*128 : +128, :] = oTn[:, i-tile]^T @ Wo
                if tail:
                    # attention is done: borrow a free score bank pair for a
                    # j-merged psum and copy out [128, 1024] in one op
                    py = psS.tile([128, LQ], F32, tag=f"sc{i % 2}", name="pyt")
                    for j in (0, 1):
                        nc.tensor.matmul(
                            py[:, j * 512:(j + 1) * 512],
                            lhsT=oT[p][:, i * 128:(i + 1) * 128],
                            rhs=woh[:, j * 512:(j + 1) * 512],
                            start=True, stop=True,
                        )
                    yt = ypool.tile([128, LQ], BF16, tag="ytl", name="ytl")
                    if i % 2 == 0:
                        nc.vector.tensor_copy(yt, py)
                    else:
                        nc.scalar.copy(yt, py)
                    nc.gpsimd.dma_start(
                        out=y[p * LQ + i * 128:p * LQ + (i + 1) * 128, :], in_=yt
                    )
                    return
                for j in (0, 1):
                    py = psA.tile([128, 512], F32, tag="ps", name="py")
                    nc.tensor.matmul(
                        py, lhsT=oT[p][:, i * 128:(i + 1) * 128],
                        rhs=woh[:, j * 512:(j + 1) * 512],
                        start=True, stop=True,
                    )
                    yt = ypool.tile([128, 512], BF16, tag="yt", name="yt")
                    nc.vector.tensor_copy(yt, py)
                    nc.gpsimd.dma_start(
                        out=y[p * LQ + i * 128:p * LQ + (i + 1) * 128,
                              j * 512:(j + 1) * 512],
                        in_=yt,
                    )

            # ---- phase 0: k block0, q half0, v block0 ----
            produce_kv(0)
            produce_q(0)

            # ---- pass 0 attention, with later block production and q half1
            # projection interleaved between tiles ----
            psoo0 = psO.tile([113, 512], F32, tag="oo", name="psoo0")
            if os.environ.get("FILLERS", "1") == "1":
                filler0 = {
                    1: lambda: produce_kv(1),
                    4: lambda: produce_kv(2),
                    7: lambda: produce_kv(3),
                    10: lambda: produce_q(1),
                }
            else:
                for lb in range(1, NLB):
                    produce_kv(lb)
                produce_q(1)
                filler0 = {}
            for t in range(16):
                attn_tile(0, psoo0, t)
                if t in filler0:
                    filler0[t]()
            normalize(0, psoo0)

            # ---- pass 1 attention; pass-0 y projection interleaved to keep
            # the PE warm while ACT crunches exp ----
            psoo1 = psO.tile([113, 512], F32, tag="oo", name="psoo1")
            for t in range(16):
                attn_tile(1, psoo1, t)
                if t % 2 == 1:
                    y_tile(0, t // 2)
            normalize(1, psoo1)
            for i in range(8):
                y_tile(1, i, tail=True)

            if dbg:
                for p in range(NP):
                    nc.sync.dma_start(out=ot_d[:, p * LQ:(p + 1) * LQ], in_=oT[p])
                    for g in range(4):
                        h, sub = g % 2, g // 2
                        c0 = p * LQ + sub * 512
                        nc.sync.dma_start(
                            out=qt_d[32 * h:32 * h + 16, c0:c0 + 512],
                            in_=qT[p][64 * sub + 32 * h:64 * sub + 32 * h + 16, :],
                        )
                        nc.sync.dma_start(
                            out=stg_d[32 * h:32 * h + 17, c0:c0 + 512],
                            in_=stg[p][32 * g:32 * g + 17, :],
                        )
                        nc.sync.dma_start(
                            out=rb_d[32 * h:32 * h + 16, c0:c0 + 512],
                            in_=rb_sb[p][32 * g:32 * g + 16, :],
                        )
                for lb in range(NLB):
                    nc.sync.dma_start(
                        out=kt_d[:, lb * LB:(lb + 1) * LB], in_=ktb[lb][0:DP, :]
                    )
                    nc.sync.dma_start(
                        out=vsb_d[:, lb * 144:(lb + 1) * 144], in_=vsb[lb]
                    )

    nc.finalize()
    return nc


def _get_nc():
    if "nc" not in _CACHE:
        _CACHE["nc"] = _build_nc()
    return _CACHE["nc"]


def _bf16(a):
    import ml_dtypes
    return np.asarray(a, dtype=np.float32).astype(ml_dtypes.bfloat16)


def _pad48(w32):
    # [*, 32] -> [*, 48] with head0 dims at 0-15, head1 at 32-47
    out = np.zeros(w32.shape[:-1] + (DP,), np.float32)
    out[..., 0:16] = w32[..., 0:16]
    out[..., 32:48] = w32[..., 16:32]
    return out


def _packw(w48):
    # [1024, 48] -> [128, CC*48]: partition p, chunk c, out-dim d
    return np.ascontiguousarray(
        w48.reshape(CC, 128, DP).transpose(1, 0, 2).reshape(128, CC * DP)
    )


def make_in_maps(queries, keys, values, Wq, bq, Wk, bk, Wv, bv, Wo, bo):
    xqt = [_bf16(queries[b].T) for b in range(B)]
    xkt = [_bf16(keys[b].T) for b in range(B)]
    xvt = [_bf16(values[b].T) for b in range(B)]
    eye = _bf16(np.eye(DP, dtype=np.float32))

    in_maps = []
    for core in range(8):
        b, hg = core // 4, core % 4
        s = DSL * hg
        wo48 = np.zeros((DP, D_MODEL), np.float32)
        wo48[0:16] = Wo[s:s + 16]
        wo48[32:48] = Wo[s + 16:s + 32]
        in_maps.append({
            "xq_t": xqt[b],
            "xk_t": xkt[b],
            "xv_t": xvt[b],
            "wq": _bf16(_packw(_pad48(Wq[:, s:s + DSL]))),
            "wk": _bf16(_packw(_pad48(Wk[:, s:s + DSL]))),
            "wv": _bf16(_packw(_pad48(Wv[:, s:s + DSL]))),
            "bqkv": np.ascontiguousarray(
                np.stack(
                    [_pad48(bq[s:s + DSL]), _pad48(bk[s:s + DSL]),
                     _pad48(bv[s:s + DSL])], axis=1
                ).astype(np.float32)
            ),
            "wo": _bf16(wo48),
            "identp": eye,
        })
    return in_maps


def kernel(queries, keys, values, Wq, bq, Wk, bk, Wv, bv, Wo, bo, **_unused):
    queries = np.asarray(queries, dtype=np.float32)
    keys = np.asarray(keys, dtype=np.float32)
    values = np.asarray(values, dtype=np.float32)
    Wq, Wk, Wv = (np.asarray(a, dtype=np.float32) for a in (Wq, Wk, Wv))
    Wo = np.asarray(Wo, dtype=np.float32)
    bq, bk, bv, bo = (np.asarray(a, dtype=np.float32) for a in (bq, bk, bv, bo))

    nc = _get_nc()
    in_maps = make_in_maps(queries, keys, values, Wq, bq, Wk, bk, Wv, bv, Wo, bo)
    res = run_bass_kernel_spmd(nc, in_maps, core_ids=list(range(8)))
    out = np.zeros((B, L, D_MODEL), np.float32)
    for core in range(8):
        out[core // 4] += np.asarray(res.results[core]["y"], dtype=np.float32)
    out += bo
    return out


# revision 33
# speedup vs baseline: 1.1374x; 1.1374x over previous
"""Multi-head attention (quirky Dense(d_k) variant) on 8 trn2 NeuronCores.

Sharding: data-parallel over batch (B=2), tensor-parallel over heads
(8 heads -> 4 groups of 2 heads). Core c: batch c//4, head-group c%4.
Each core is fully independent (no collectives); host sums the 4 partial
outputs per batch (Wo row-sharded -> partial sums) and adds bo.

All matmul operands are bf16 (f32 PSUM accumulation); inputs/weights are
converted to bf16 on the host, halving HBM traffic and running the PE at
full rate. The two heads' 16 dims sit at partitions 0-15 and 32-47
(32-aligned bases) so score matmuls row-tile-pack (tile_position (0,0) /
(32,0)); k/v and q-half projections col-tile-pack (out partitions 0-47 /
64-111 of one PSUM bank) so two projection streams run concurrently.

Per-core dataflow (L=2048, d_model=1024), two passes over q halves:
  qT/kT/vT = W48^T @ X^T        (PE bf16 col-packed; bias folded into the
                                 DVE PSUM->SBUF copy as tensor_scalar_add)
  v        = transpose(vT)      (PE transpose, per 128-key tile)
  s        = k_h^T . q_h        (PE, [128 k x 1024 q], 2 heads row-packed)
  e        = exp(s * scale)     (ACT, PSUM f32 -> SBUF bf16; no max sub)
  oT/rsum  = [v|1]^T @ e        (PE, PSUM [49, 1024], rsum via ones col)
  rrec     = 1/rsum             (DMA transpose to [128,16], DVE reciprocal,
                                 DMA back to rows, PE K=1 broadcast to rb)
  oTn      = oT * rb            (DVE, normalized bf16 oT; rows 16-31 zero)
  y        = oTn^T @ Wo         (PE K=48; Wo rows 16-31 zero on host)
"""

import math
import sys

sys.path.insert(0, "/opt/trn_rl_repo")

import numpy as np

import concourse.bass as bass
import concourse.mybir as mybir
import concourse.tile as tile
from concourse import bacc
from concourse.bass_utils import run_bass_kernel_spmd

H = 8
D_MODEL = 1024
D_K = 128          # projection width (d_model / h)
HD = 16            # per-head dim after reshape
B, L = 2, 2048
DSL = 32           # per-core slice of D_K (2 heads x 16)
DP = 48            # padded: head0 dims at 0-15, head1 at 32-47
CC = 8             # contraction chunks of 128 over d_model
LQ = 1024          # queries per pass
NP = 2             # q passes
NLB = 4            # k/v blocks
LB = L // NLB      # 512
SCALE = 1.0 / math.sqrt(float(D_K))   # reference scales by sqrt(d_k)=sqrt(128)
F32 = mybir.dt.float32
BF16 = mybir.dt.bfloat16

_CACHE = {}


def _build_nc():
    nc = bacc.Bacc(None, target_bir_lowering=False)

    xq = nc.declare_dram_parameter("xq_t", [D_MODEL, L], BF16, isOutput=False)
    xk = nc.declare_dram_parameter("xk_t", [D_MODEL, L], BF16, isOutput=False)
    xv = nc.declare_dram_parameter("xv_t", [D_MODEL, L], BF16, isOutput=False)
    # host-packed [128, CC*DP]: partition p, chunk c, out-dim d
    wq = nc.declare_dram_parameter("wq", [128, CC * DP], BF16, isOutput=False)
    wk = nc.declare_dram_parameter("wk", [128, CC * DP], BF16, isOutput=False)
    wv = nc.declare_dram_parameter("wv", [128, CC * DP], BF16, isOutput=False)
    bqkv = nc.declare_dram_parameter("bqkv", [DP, 3], F32, isOutput=False)
    # rows 0-15: Wo head0; 32-47: Wo head1; 16-31 zero (bo added on host)
    wo = nc.declare_dram_parameter("wo", [DP, D_MODEL], BF16, isOutput=False)
    identp = nc.declare_dram_parameter("identp", [DP, DP], BF16, isOutput=False)
    y = nc.declare_dram_parameter("y", [L, D_MODEL], BF16, isOutput=True)
    import os
    dbg = os.environ.get("KERNEL_DEBUG", "0") == "1"
    if dbg:
        qt_d = nc.declare_dram_parameter("qt_d", [DP, L], BF16, isOutput=True)
        kt_d = nc.declare_dram_parameter("kt_d", [DP, L], BF16, isOutput=True)
        vsb_d = nc.declare_dram_parameter("vsb_d", [128, 4 * 144], BF16, isOutput=True)
        ot_d = nc.declare_dram_parameter("ot_d", [DP, L], BF16, isOutput=True)
        stg_d = nc.declare_dram_parameter("stg_d", [49, L], F32, isOutput=True)
        rb_d = nc.declare_dram_parameter("rb_d", [DP, L], F32, isOutput=True)
        e_d = nc.declare_dram_parameter("e_d", [128, 4 * LQ], BF16, isOutput=True)
        sc_d = nc.declare_dram_parameter("sc_d", [128, 4 * LQ], F32, isOutput=True)

    Exp = mybir.ActivationFunctionType.Exp
    Mult = mybir.AluOpType.mult

    with tile.TileContext(nc) as tc:
        with (
            tc.tile_pool(name="const", bufs=1) as constp,
            tc.tile_pool(name="xin", bufs=1) as xpool,
            tc.tile_pool(name="qk", bufs=1) as qkpool,
            tc.tile_pool(name="psS", bufs=1, space="PSUM") as psS,
            tc.tile_pool(name="psO", bufs=2, space="PSUM") as psO,
            tc.tile_pool(name="psA", bufs=2, space="PSUM") as psA,
            tc.tile_pool(name="ep", bufs=2) as epool,
            tc.tile_pool(name="yp", bufs=3) as ypool,
            tc.tile_pool(name="misc", bufs=2) as misc,
            tc.tile_pool(name="dr", bufs=2, space="DRAM") as drpool,
        ):
            # ---- constants (gpsimd DMA queue; x loads go on sync) ----
            wq_sb = constp.tile([128, CC, DP], BF16)
            nc.gpsimd.dma_start(out=wq_sb, in_=wq[:].rearrange("p (c d) -> p c d", c=CC))
            wk_sb = constp.tile([128, CC, DP], BF16)
            nc.gpsimd.dma_start(out=wk_sb, in_=wk[:].rearrange("p (c d) -> p c d", c=CC))
            wv_sb = constp.tile([128, CC, DP], BF16)
            nc.gpsimd.dma_start(out=wv_sb, in_=wv[:].rearrange("p (c d) -> p c d", c=CC))
            woh = constp.tile([DP, D_MODEL], BF16)
            nc.gpsimd.dma_start(out=woh, in_=wo[:])
            bias_sb = constp.tile([DP, 3], F32)
            nc.gpsimd.dma_start(out=bias_sb, in_=bqkv[:])
            ident = constp.tile([DP, DP], BF16)
            nc.gpsimd.dma_start(out=ident, in_=identp[:])
            # force the ACT exp table-set load off the critical path
            warm = constp.tile([1, 16], F32)
            nc.vector.memset(warm, 0.0)
            warm2 = constp.tile([1, 16], F32)
            nc.scalar.activation(warm2, warm, Exp, scale=1.0)

            # ---- resident x: one DMA per (tensor, L-half): [128, 8, 1024]
            # gathers have 2KB/partition contiguous lines (512 descriptors),
            # and only 1MB gates the first projections of each half ----
            xkh = [xpool.tile([128, CC, LQ], BF16, name=f"xkh{_i}") for _i in range(2)]
            xvh = [xpool.tile([128, CC, LQ], BF16, name=f"xvh{_i}") for _i in range(2)]
            xqh = [xpool.tile([128, CC, LQ], BF16, name=f"xqh{_i}") for _i in range(2)]

            def load_x(dram, sb, hf, eng):
                eng.dma_start(
                    out=sb,
                    in_=dram[:, hf * LQ:(hf + 1) * LQ]
                    .rearrange("(c p) l -> p c l", p=128),
                )

            load_x(xk, xkh[0], 0, nc.sync)
            load_x(xq, xqh[0], 0, nc.scalar)
            load_x(xv, xvh[0], 0, nc.gpsimd)
            load_x(xk, xkh[1], 1, nc.sync)
            load_x(xv, xvh[1], 1, nc.scalar)
            load_x(xq, xqh[1], 1, nc.sync)

            class _XH:
                # x[cc][:, c0:c0+w] view over the two L-half tiles
                def __init__(self, halves):
                    self.halves = halves
                def slice(self, cc, c0, w):
                    hf = c0 // LQ
                    return self.halves[hf][:, cc, c0 - hf * LQ:c0 - hf * LQ + w]

            xkc, xvc, xqc = _XH(xkh), _XH(xvh), _XH(xqh)

            # qT4: col-packed projection layout kept as-is -- q-sub0 dims at
            # rows 0-47, q-sub1 at rows 64-111 (4 head/sub groups at 32k)
            qT = [qkpool.tile([112, LB], BF16, name=f"qT{_i}") for _i in range(NP)]
            # ktb4: k^T at rows 0-47 plus a copy at rows 64-111 for the
            # 4-way row-packed score matmuls
            ktb = [qkpool.tile([112, LB], BF16, name=f"ktb{_i}") for _i in range(NLB)]
            vtb = [qkpool.tile([DP, LB], BF16, name=f"vtb{_i}") for _i in range(NLB)]
            # v natural + ones col, per Lk-tile t within block (36 cols):
            # [16 v_h0 | 1 | pad | 16 v_h1 | 1 | pad]
            vsb = [constp.tile([128, 4 * 36], BF16, name=f"vsb{_i}") for _i in range(NLB)]
            for lb in range(NLB):
                v3 = vsb[lb].rearrange("p (t s) -> p t s", s=36)
                nc.vector.memset(v3[:, :, 16:17], 1.0)
                nc.vector.memset(v3[:, :, 34:35], 1.0)

            # oTn rows 0-15 / 32-47: normalized o^T per head (bf16);
            # rows 16-31 zeroed so the K=48 output matmul is junk-safe
            oT = [qkpool.tile([DP, LQ], BF16, name=f"oT{_i}") for _i in range(NP)]
            for p in range(NP):
                nc.vector.memset(oT[p], 0.0)
            stg = [misc.tile([113, 512], F32, tag=f"stg{_i}", name=f"stg{_i}")
                   for _i in range(NP)]
            rb_sb = [misc.tile([128, 512], F32, tag=f"rb{_i}", name=f"rb{_i}")
                     for _i in range(NP)]
            scratch = [drpool.tile([4, 512], F32, name=f"scr{_i}") for _i in range(NP)]

            import os

            def proj_pair(w0, x0, c0, out0, b0, w1, x1, c1, out1, b1):
                # two 48-wide projection streams col-packed in one PSUM bank:
                # stream0 -> partitions 0-47, stream1 -> partitions 64-111
                pb = psA.tile([112, LB], F32, tag="ps", name="pb")
                for cc in range(CC):
                    nc.tensor.matmul(
                        pb[0:DP, :], lhsT=w0[:, cc, :], rhs=x0.slice(cc, c0, LB),
                        start=(cc == 0), stop=(cc == CC - 1),
                    )
                    nc.tensor.matmul(
                        pb[64:64 + DP, :], lhsT=w1[:, cc, :], rhs=x1.slice(cc, c1, LB),
                        start=(cc == 0), stop=(cc == CC - 1),
                    )
                nc.vector.tensor_scalar_add(out0, pb[0:DP, :], bias_sb[:, b0:b0 + 1])
                nc.vector.tensor_scalar_add(out1, pb[64:64 + DP, :], bias_sb[:, b1:b1 + 1])

            def produce_kv(lb):
                proj_pair(
                    wk_sb, xkc, lb * LB, ktb[lb][0:DP, :], 1,
                    wv_sb, xvc, lb * LB, vtb[lb], 2,
                )
                nc.gpsimd.tensor_copy(ktb[lb][64:64 + DP, :], ktb[lb][0:DP, :])
                # transpose this block's v into natural layout
                pvt = psA.tile([128, 4 * DP], BF16, tag="ps", name=f"pvt{lb}")
                for i in range(4):
                    nc.tensor.transpose(
                        pvt[:, i * DP:(i + 1) * DP],
                        vtb[lb][:, i * 128:(i + 1) * 128],
                        ident,
                    )
                v3 = vsb[lb].rearrange("p (t s) -> p t s", s=36)
                pvt3 = pvt.rearrange("p (t s) -> p t s", s=DP)
                for h in (0, 1):
                    nc.vector.tensor_copy(
                        v3[:, :, 18 * h:18 * h + 16], pvt3[:, :, 32 * h:32 * h + 16]
                    )

            def produce_q(p):
                proj_pair(
                    wq_sb, xqc, p * LQ, qT[p][0:DP, :], 0,
                    wq_sb, xqc, p * LQ + 512, qT[p][64:64 + DP, :], 0,
                )

            def attn_tile(p, psoo, t):
                lb, ti = t // 4, t % 4
                sc, es = [], []
                for h in (0, 1):
                    sc.append(psS.tile([128, LQ], F32, tag=f"sc{h}", name=f"sc{h}"))
                    es.append(epool.tile([128, LQ], BF16, tag=f"e{h}", name=f"et{h}"))
                # 4 score matmuls run concurrently in 4 PE row groups:
                # group g = 2*sub + h at rows 32g (q/k copies pre-placed)
                for sub in (0, 1):
                    for h in (0, 1):
                        g = 64 * sub + 32 * h
                        nc.tensor.matmul(
                            sc[h][:, sub * 512:(sub + 1) * 512],
                            lhsT=ktb[lb][g:g + 16, ti * 128:(ti + 1) * 128],
                            rhs=qT[p][g:g + 16, :],
                            start=True, stop=True,
                            tile_position=(g, 0),
                        )
                for h in (0, 1):
                    nc.scalar.activation(es[h], sc[h], Exp, scale=SCALE)
                if dbg and t == 15:
                    for h in (0, 1):
                        off = (2 * p + h) * LQ
                        nc.sync.dma_start(out=e_d[:, off:off + LQ], in_=es[h])
                        cpt = misc.tile([128, LQ], F32, tag="scd", name="scd")
                        nc.vector.tensor_copy(cpt, sc[h])
                        nc.sync.dma_start(out=sc_d[:, off:off + LQ], in_=cpt)
                # 4 o-accum matmuls in 4 PE col groups into one PSUM bank:
                # psoo rows 32g..32g+16 for group g = 2*sub + h
                for h in (0, 1):
                    for sub in (0, 1):
                        g = 64 * sub + 32 * h
                        nc.tensor.matmul(
                            psoo[g:g + 17, :],
                            lhsT=vsb[lb][:, ti * 36 + 18 * h:ti * 36 + 18 * h + 17],
                            rhs=es[h][:, sub * 512:(sub + 1) * 512],
                            start=(t == 0), stop=(t == 15),
                            tile_position=(0, g),
                        )

            def normalize(p, psoo):
                # stage PSUM rows to SBUF f32 (frees psoo for the next pass);
                # one copy on ACT, one on DVE so they run concurrently
                nc.scalar.copy(stg[p][0:49, :], psoo[0:49, :])
                nc.vector.tensor_copy(stg[p][64:113, :], psoo[64:113, :])
                # rsum rows (16+32g) -> DRAM -> [16, 128] -> reciprocal
                for g in range(4):
                    nc.gpsimd.dma_start(
                        out=scratch[p][g:g + 1, :], in_=stg[p][32 * g + 16:32 * g + 17, :]
                    )
                rsT = misc.tile([16, 128], F32, tag="rsT", name="rsT")
                nc.sync.dma_start(
                    out=rsT, in_=scratch[p][:].rearrange("g (i p) -> (g i) p", p=128)
                )
                rrT = misc.tile([16, 128], F32, tag="rrT", name="rrT")
                nc.vector.reciprocal(rrT, rsT)
                # back to DRAM, then stride-0 broadcast to 16 partitions/group
                scr3 = drpool.tile([4, 512], F32, name=f"scr3_{p}")
                nc.gpsimd.dma_start(
                    out=scr3[:].rearrange("g (i p) -> (g i) p", p=128), in_=rrT
                )
                for g in range(4):
                    eng = nc.sync if g % 2 else nc.gpsimd
                    eng.dma_start(
                        out=rb_sb[p][32 * g:32 * g + 16, :],
                        in_=scr3[g:g + 1, :].to_broadcast((16, 512)),
                    )
                # normalized bf16 oT for the output projection; group g holds
                # head g%2, q-sub g//2 -> oT[32h:+16, 512*sub:+512]
                for g in range(4):
                    h, sub = g % 2, g // 2
                    eng = nc.vector if g % 2 else nc.gpsimd
                    eng.tensor_tensor(
                        out=oT[p][32 * h:32 * h + 16, sub * 512:(sub + 1) * 512],
                        in0=stg[p][32 * g:32 * g + 16, :],
                        in1=rb_sb[p][32 * g:32 * g + 16, :],
                        op=Mult,
                    )

            def y_tile(p, i, tail=False):
                # y[p*1024 + i*128 : +128, :] = oTn[:, i-tile]^T @ Wo
                if tail:
                    # attention is done: borrow a free score bank pair for a
                    # j-merged psum and copy out [128, 1024] in one op
                    py = psS.tile([128, LQ], F32, tag=f"sc{i % 2}", name="pyt")
                    for j in (0, 1):
                        nc.tensor.matmul(
                            py[:, j * 512:(j + 1) * 512],
                            lhsT=oT[p][:, i * 128:(i + 1) * 128],
                            rhs=woh[:, j * 512:(j + 1) * 512],
                            start=True, stop=True,
                        )
                    yt = ypool.tile([128, LQ], BF16, tag="ytl", name="ytl")
                    if i % 2 == 0:
                        nc.vector.tensor_copy(yt, py)
                    else:
                        nc.scalar.copy(yt, py)
                    eng = nc.gpsimd if i % 2 else nc.sync
                    eng.dma_start(
                        out=y[p * LQ + i * 128:p * LQ + (i + 1) * 128, :], in_=yt
                    )
                    return
                for j in (0, 1):
                    py = psA.tile([128, 512], F32, tag="ps", name="py")
                    nc.tensor.matmul(
                        py, lhsT=oT[p][:, i * 128:(i + 1) * 128],
                        rhs=woh[:, j * 512:(j + 1) * 512],
                        start=True, stop=True,
                    )
                    yt = ypool.tile([128, 512], BF16, tag="yt", name="yt")
                    nc.vector.tensor_copy(yt, py)
                    nc.gpsimd.dma_start(
                        out=y[p * LQ + i * 128:p * LQ + (i + 1) * 128,
                              j * 512:(j + 1) * 512],
                        in_=yt,
                    )

            # ---- phase 0: k block0, q half0, v block0 ----
            produce_kv(0)
            produce_q(0)

            # ---- pass 0 attention, with later block production and q half1
            # projection interleaved between tiles ----
            psoo0 = psO.tile([113, 512], F32, tag="oo", name="psoo0")
            if os.environ.get("FILLERS", "1") == "1":
                filler0 = {
                    1: lambda: produce_kv(1),
                    4: lambda: produce_kv(2),
                    7: lambda: produce_kv(3),
                    10: lambda: produce_q(1),
                }
            else:
                for lb in range(1, NLB):
                    produce_kv(lb)
                produce_q(1)
                filler0 = {}
            for t in range(16):
                attn_tile(0, psoo0, t)
                if t in filler0:
                    filler0[t]()
            normalize(0, psoo0)

            # ---- pass 1 attention; pass-0 y projection interleaved to keep
            # the PE warm while ACT crunches exp ----
            psoo1 = psO.tile([113, 512], F32, tag="oo", name="psoo1")
            for t in range(16):
                attn_tile(1, psoo1, t)
                if t % 2 == 1:
                    y_tile(0, t // 2)
            normalize(1, psoo1)
            for i in range(8):
                y_tile(1, i, tail=True)

            if dbg:
                for p in range(NP):
                    nc.sync.dma_start(out=ot_d[:, p * LQ:(p + 1) * LQ], in_=oT[p])
                    for g in range(4):
                        h, sub = g % 2, g // 2
                        c0 = p * LQ + sub * 512
                        nc.sync.dma_start(
                            out=qt_d[32 * h:32 * h + 16, c0:c0 + 512],
                            in_=qT[p][64 * sub + 32 * h:64 * sub + 32 * h + 16, :],
                        )
                        nc.sync.dma_start(
                            out=stg_d[32 * h:32 * h + 17, c0:c0 + 512],
                            in_=stg[p][32 * g:32 * g + 17, :],
                        )
                        nc.sync.dma_start(
                            out=rb_d[32 * h:32 * h + 16, c0:c0 + 512],
                            in_=rb_sb[p][32 * g:32 * g + 16, :],
                        )
                for lb in range(NLB):
                    nc.sync.dma_start(
                        out=kt_d[:, lb * LB:(lb + 1) * LB], in_=ktb[lb][0:DP, :]
                    )
                    nc.sync.dma_start(
                        out=vsb_d[:, lb * 144:(lb + 1) * 144], in_=vsb[lb]
                    )

    nc.finalize()
    return nc


def _get_nc():
    if "nc" not in _CACHE:
        _CACHE["nc"] = _build_nc()
    return _CACHE["nc"]


def _bf16(a):
    import ml_dtypes
    return np.asarray(a, dtype=np.float32).astype(ml_dtypes.bfloat16)


def _pad48(w32):
    # [*, 32] -> [*, 48] with head0 dims at 0-15, head1 at 32-47
    out = np.zeros(w32.shape[:-1] + (DP,), np.float32)
    out[..., 0:16] = w32[..., 0:16]
    out[..., 32:48] = w32[..., 16:32]
    return out


def _packw(w48):
    # [1024, 48] -> [128, CC*48]: partition p, chunk c, out-dim d
    return np.ascontiguousarray(
        w48.reshape(CC, 128, DP).transpose(1, 0, 2).reshape(128, CC * DP)
    )


def make_in_maps(queries, keys, values, Wq, bq, Wk, bk, Wv, bv, Wo, bo):
    xqt = [_bf16(queries[b].T) for b in range(B)]
    xkt = [_bf16(keys[b].T) for b in range(B)]
    xvt = [_bf16(values[b].T) for b in range(B)]
    eye = _bf16(np.eye(DP, dtype=np.float32))

    in_maps = []
    for core in range(8):
        b, hg = core // 4, core % 4
        s = DSL * hg
        wo48 = np.zeros((DP, D_MODEL), np.float32)
        wo48[0:16] = Wo[s:s + 16]
        wo48[32:48] = Wo[s + 16:s + 32]
        in_maps.append({
            "xq_t": xqt[b],
            "xk_t": xkt[b],
            "xv_t": xvt[b],
            "wq": _bf16(_packw(_pad48(Wq[:, s:s + DSL]))),
            "wk": _bf16(_packw(_pad48(Wk[:, s:s + DSL]))),
            "wv": _bf16(_packw(_pad48(Wv[:, s:s + DSL]))),
            "bqkv": np.ascontiguousarray(
                np.stack(
                    [_pad48(bq[s:s + DSL]), _pad48(bk[s:s + DSL]),
                     _pad48(bv[s:s + DSL])], axis=1
                ).astype(np.float32)
            ),
            "wo": _bf16(wo48),
            "identp": eye,
        })
    return in_maps


def kernel(queries, keys, values, Wq, bq, Wk, bk, Wv, bv, Wo, bo, **_unused):
    queries = np.asarray(queries, dtype=np.float32)
    keys = np.asarray(keys, dtype=np.float32)
    values = np.asarray(values, dtype=np.float32)
    Wq, Wk, Wv = (np.asarray(a, dtype=np.float32) for a in (Wq, Wk, Wv))
    Wo = np.asarray(Wo, dtype=np.float32)
    bq, bk, bv, bo = (np.asarray(a, dtype=np.float32) for a in (bq, bk, bv, bo))

    nc = _get_nc()
    in_maps = make_in_maps(queries, keys, values, Wq, bq, Wk, bk, Wv, bv, Wo, bo)
    res = run_bass_kernel_spmd(nc, in_maps, core_ids=list(range(8)))
    out = np.zeros((B, L, D_MODEL), np.float32)
    for core in range(8):
        out[core // 4] += np.asarray(res.results[core]["y"], dtype=np.float32)
    out += bo
    return out
